# revision 10
# baseline (speedup 1.0000x reference)
"""Behler G3 symmetry-function kernel for Trainium2 (8 NeuronCores).

Math (per batch b, atom n; reduction over triples t):
    fc(r)      = 0.5*(cos(pi*r/6)+1) = sin(pi*r/12 + pi/2)^2        (r < 6 always)
    u          = r_ij^2 + r_ik^2
    1 - cos_t  = (r_jk^2 - (r_ij-r_ik)^2) / (2 r_ij r_ik)
               = numer2 / (2 p),  numer2 = 2p + (r_jk^2 - u), p = r_ij r_ik
    xq         = (1-cos_t)/2 = numer2 * (1/p) * 0.25                 in [0,1]
    R          = fc(r_ij)*fc(r_ik)
    G_z        = R * xq^z                       z in {1,2,4,16}
    E_e        = exp(-eta_e * u)                e in 0..7
    S[n,e,z]   = sum_t E_e * G_z
    out[n, e*8+a] = 2*S[e,a]              for a<4
                  = 2^(1+2*z)*S[e,a-4]    for a>=4   (z = zeta[a-4])
  (reference ang coeffs 2^(1±z) on (1-cos)^z equal these on xq^z.)

Sharding: data-parallel over batch: core b handles batch b. No collectives.

Host-side prep inside kernel(): the t-reduction is permutation-invariant, so
triples are compacted by mask per (b,n) — valid triples first, padded to the
max valid count (T'). Padding entries use r=6.0, where fc(6)=0 exactly, so
they contribute nothing; the mask tensor never ships to the device.

Eta values and T' are baked into the program at build time (the program is
rebuilt per kernel() call, so any inputs work).
"""

import math
import os
import sys

import numpy as np

if "/opt/trn_rl_repo" not in sys.path:
    sys.path.insert(0, "/opt/trn_rl_repo")

from contextlib import ExitStack

import concourse.bass as bass
import concourse.tile as tile
from concourse import bacc, mybir
from concourse.bass_utils import run_bass_kernel_spmd

F32 = mybir.dt.float32
F16 = mybir.dt.float16
I32 = mybir.dt.int32
Act = mybir.ActivationFunctionType
Alu = mybir.AluOpType

B, N, T = 8, 512, 512
P = 128                    # SBUF partitions
NCH = N // P               # 4 n-chunks
ZETAS = (1, 2, 4, 16)
NE = 8                     # etas
NZ = 4

# dtype of the contraction inputs (E and G tiles). f16 doubles the DVE
# product throughput; error ~3e-4 of absmax. F32 is the safe mode.
PROD_DT = F16

# Contraction split over the 32 (e,z) pairs:
#   first DVE_PAIRS -> fused scalar_tensor_tensor+accum on DVE
#   next POOLPROD_PAIRS -> GpSimd f16 product + ACT Copy-with-accum reduce
#   rest            -> DVE f16 product + ACT Copy-with-accum reduce
DVE_PAIRS = int(os.environ.get("BEHLER_DVE_PAIRS", "14"))
POOLPROD_PAIRS = int(os.environ.get("BEHLER_POOLPROD_PAIRS", "13"))

# Engine per square-family op: "act" | "dve" | "gps".
SQ_ENGINES = {
    "fij": "act", "fik": "act",            # fc = sin^2
    "sqij": "act", "sqik": "act", "sqjk": "act",
    "x2": "dve", "x4": "dve", "x8": "dve", "x16": "dve",
}


def _build_nc(etas: np.ndarray, Tp: int) -> bass.Bass:
    W = NCH * Tp
    nc = bacc.Bacc("TRN2", target_bir_lowering=False, debug=False, num_devices=B)

    d_rij = nc.dram_tensor("r_ij", [N, Tp], F32, kind="ExternalInput").ap()
    d_rik = nc.dram_tensor("r_ik", [N, Tp], F32, kind="ExternalInput").ap()
    d_rjk = nc.dram_tensor("r_jk", [N, Tp], F32, kind="ExternalInput").ap()
    d_out = nc.dram_tensor("out", [N, NE * 2 * NZ], F32, kind="ExternalOutput").ap()

    # Register the Sin bias constant (pi/2) like Bass.__init__ does for 0/1.
    half_pi = math.pi / 2
    _const_t = nc.alloc_sbuf_tensor("const-f32-half-pi", [128, 1], F32)
    nc.gpsimd.memset(_const_t.ap(), half_pi)
    nc.const_aps.aps[(F32, half_pi)] = _const_t.ap()
    nc.all_engine_barrier()

    with tile.TileContext(nc) as tc, ExitStack() as ctx:
        pool = ctx.enter_context(tc.tile_pool(name="main", bufs=1))

        # tags are physical slots (reserved per tag for the pool's
        # lifetime); tensors with disjoint lifetimes share a slot.
        def mega(slot, sem_name, dt=F32):
            return pool.tile([P, W], dt, tag=slot, name=sem_name)

        def square(dst, src, eng):
            if eng == "act":
                nc.scalar.activation(dst[:], src[:], Act.Square)
            elif eng == "dve":
                nc.vector.tensor_mul(dst[:], src[:], src[:])
            else:
                nc.gpsimd.tensor_mul(dst[:], src[:], src[:])

        # ---- load inputs: chunk c of DRAM rows -> mega cols [c*Tp,(c+1)*Tp) ----
        rij = mega("s0", "rij")
        rik = mega("s1", "rik")
        rjk = mega("s2", "rjk")
        for tl, dr in ((rij, d_rij), (rik, d_rik), (rjk, d_rjk)):
            for c in range(NCH):
                nc.sync.dma_start(
                    out=tl[:, c * Tp:(c + 1) * Tp], in_=dr[c * P:(c + 1) * P, :]
                )

        # ---- fc via sin^2 (ACT, sin table set first) ----
        fijs = mega("s3", "fijs")
        fiks = mega("s4", "fiks")
        nc.scalar.activation(fijs[:], rij[:], Act.Sin,
                             bias=math.pi / 2, scale=math.pi / 12)
        nc.scalar.activation(fiks[:], rik[:], Act.Sin,
                             bias=math.pi / 2, scale=math.pi / 12)
        fij = mega("s5", "fij")
        fik = mega("s6", "fik")
        square(fij, fijs, SQ_ENGINES["fij"])
        square(fik, fiks, SQ_ENGINES["fik"])

        # ---- squares / u / p / numer2 / xq ----
        sqij = mega("s7", "sqij")
        sqik = mega("s8", "sqik")
        sqjk = mega("s9", "sqjk")
        square(sqij, rij, SQ_ENGINES["sqij"])
        square(sqik, rik, SQ_ENGINES["sqik"])
        square(sqjk, rjk, SQ_ENGINES["sqjk"])

        p = mega("s10", "p")
        nc.vector.tensor_mul(p[:], rij[:], rik[:])       # rij, rik dead
        u = mega("s11", "u")
        nc.vector.tensor_add(u[:], sqij[:], sqik[:])     # sqij, sqik dead
        tsub = mega("s7", "tsub")
        nc.vector.tensor_sub(tsub[:], sqjk[:], u[:])     # sqjk dead

        rp = mega("s8", "rp")
        rscr = mega("s3", "rscr")                        # fijs dead
        nc.vector.reciprocal_approx_accurate(out=rp[:], in_=p[:], scratch=rscr[:])

        numer2 = mega("s0", "numer2")
        nc.vector.scalar_tensor_tensor(
            numer2[:], p[:], 2.0, tsub[:], op0=Alu.mult, op1=Alu.add
        )                                                # p, tsub dead
        xq = mega("s1", "xq")
        nc.vector.scalar_tensor_tensor(
            xq[:], rp[:], 0.25, numer2[:], op0=Alu.mult, op1=Alu.mult
        )                                                # rp, numer2 dead

        R = mega("s2", "R")
        nc.vector.tensor_mul(R[:], fij[:], fik[:])       # fij, fik dead

        # ---- xq powers ----
        x2 = mega("s4", "x2")                            # fiks dead
        x4 = mega("s9", "x4")
        x8 = mega("s10", "x8")                           # p dead
        x16 = mega("s7", "x16")                          # tsub dead
        square(x2, xq, SQ_ENGINES["x2"])
        square(x4, x2, SQ_ENGINES["x4"])
        square(x8, x4, SQ_ENGINES["x8"])
        square(x16, x8, SQ_ENGINES["x16"])

        # ---- G_z = R * xq^z  (gpsimd; f16 out) ----
        powers = {1: xq, 2: x2, 4: x4, 16: x16}
        G = {}
        for z in ZETAS:
            G[z] = mega(f"g{z}", f"g{z}", PROD_DT)
            nc.gpsimd.tensor_mul(G[z][:], R[:], powers[z][:])

        # ---- E_e = exp(-eta_e * u)  (ACT, exp table set; f16 out) ----
        E = []
        for e in range(NE):
            te = mega(f"e{e}", f"e{e}", PROD_DT)
            nc.scalar.activation(te[:], u[:], Act.Exp, scale=-float(etas[e]))
            E.append(te)

        # ---- contraction: S[n, c*32 + e*4 + zi] = sum_t E_e*G_z ----
        S = pool.tile([P, NCH * NE * NZ], F32, tag="S", name="S")
        scr_d = pool.tile([P, Tp], PROD_DT, tag="scr_d", name="scr_d")
        scr_a = pool.tile([P, Tp], PROD_DT, tag="scr_a", name="scr_a")

        pairs = [(e, zi) for e in range(NE) for zi in range(NZ)]
        for pi, (e, zi) in enumerate(pairs):
            z = ZETAS[zi]
            if pi < DVE_PAIRS:
                method = "dve"
            elif pi < DVE_PAIRS + POOLPROD_PAIRS:
                method = "actp"      # pool product + ACT accum
            else:
                method = "actd"      # dve product + ACT accum
            if method != "dve":
                prod = pool.tile([P, W], PROD_DT, tag=f"prod_{method}",
                                 name=f"prod{pi}", bufs=3)
                if method == "actp":
                    nc.gpsimd.tensor_mul(prod[:], E[e][:], G[z][:])
                else:
                    nc.vector.tensor_mul(prod[:], E[e][:], G[z][:])
            for c in range(NCH):
                sl = slice(c * Tp, (c + 1) * Tp)
                col = c * (NE * NZ) + e * NZ + zi
                acc = S[:, col:col + 1]
                if method == "dve":
                    nc.vector.scalar_tensor_tensor(
                        scr_d[:], E[e][:, sl], 1.0, G[z][:, sl],
                        op0=Alu.mult, op1=Alu.mult, accum_out=acc,
                    )
                else:
                    nc.scalar.activation(
                        scr_a[:], prod[:, sl], Act.Copy, accum_out=acc,
                    )

        # ---- epilogue: out[n, e*8+a], a<4: 2*S ; a>=4: 2^(1+2z)*S ----
        out64 = pool.tile([P, NCH * NE * 2 * NZ], F32, tag="out64", name="out64")
        S_v = S[:].rearrange("p (c e z) -> p c e z", c=NCH, e=NE, z=NZ)
        o_v = out64[:].rearrange("p (c e a) -> p c e a", c=NCH, e=NE, a=2 * NZ)
        for zi, z in enumerate(ZETAS):
            nc.vector.tensor_scalar_mul(o_v[:, :, :, zi], S_v[:, :, :, zi], 2.0)
            nc.vector.tensor_scalar_mul(
                o_v[:, :, :, 4 + zi], S_v[:, :, :, zi], float(2.0 ** (1 + 2 * z))
            )

        for c in range(NCH):
            nc.sync.dma_start(
                out=d_out[c * P:(c + 1) * P, :],
                in_=out64[:, c * (2 * NE * NZ):(c + 1) * (2 * NE * NZ)],
            )

    nc.compile()
    return nc


def _prepare(r_ij, r_ik, r_jk, mask_triples):
    """Compact triples by mask per (b,n); pad to T' with fc-killing r=6."""
    valid = mask_triples != 0
    counts = valid.sum(-1)
    Tp = int(counts.max())
    Tp = max(64, ((Tp + 63) // 64) * 64)
    Tp = min(Tp, T)
    order = np.argsort(~valid, axis=-1, kind="stable")[..., :Tp]
    take = lambda a: np.ascontiguousarray(
        np.take_along_axis(np.asarray(a, dtype=np.float32), order, axis=-1))
    rij, rik, rjk = take(r_ij), take(r_ik), take(r_jk)
    pad = ~np.take_along_axis(valid, order, axis=-1)
    rij[pad] = 6.0
    rik[pad] = 6.0
    rjk[pad] = 6.0
    return rij, rik, rjk, Tp


def kernel(r_ij, r_ik, r_jk, mask_triples, etas):
    mask = np.asarray(mask_triples)
    etas = np.asarray(etas, dtype=np.float32)

    rij, rik, rjk, Tp = _prepare(r_ij, r_ik, r_jk, mask)
    nc = _build_nc(etas, Tp)
    in_maps = [
        {"r_ij": rij[b], "r_ik": rik[b], "r_jk": rjk[b]} for b in range(B)
    ]
    res = run_bass_kernel_spmd(
        nc,
        in_maps,
        core_ids=list(range(B)),
        trace=bool(int(os.environ.get("BEHLER_TRACE", "0"))),
    )
    out = np.stack([res.results[b]["out"] for b in range(B)]).astype(np.float32)
    if getattr(kernel, "_keep_results", False):
        kernel._last_results = res
    return out


# revision 12
# speedup vs baseline: 1.0006x; 1.0006x over previous
"""Behler G3 symmetry-function kernel for Trainium2 (8 NeuronCores).

Math (per batch b, atom n; reduction over triples t):
    fc(r)      = 0.5*(cos(pi*r/6)+1) = sin(pi*r/12 + pi/2)^2        (r < 6 always)
    u          = r_ij^2 + r_ik^2
    1 - cos_t  = (r_jk^2 - (r_ij-r_ik)^2) / (2 r_ij r_ik)
               = numer2 / (2 p),  numer2 = 2p + (r_jk^2 - u), p = r_ij r_ik
    xq         = (1-cos_t)/2 = numer2 * (1/p) * 0.25                 in [0,1]
    R          = fc(r_ij)*fc(r_ik)
    G_z        = R * xq^z                       z in {1,2,4,16}
    E_e        = exp(-eta_e * u)                e in 0..7
    S[n,e,z]   = sum_t E_e * G_z
    out[n, e*8+a] = 2*S[e,a]              for a<4
                  = 2^(1+2*z)*S[e,a-4]    for a>=4   (z = zeta[a-4])
  (reference ang coeffs 2^(1±z) on (1-cos)^z equal these on xq^z.)

Sharding: data-parallel over batch: core b handles batch b. No collectives.

Host-side prep inside kernel(): the t-reduction is permutation-invariant, so
triples are compacted by mask per (b,n) — valid triples first, padded to the
max valid count (T'). Padding entries use r=6.0, where fc(6)=0 exactly, so
they contribute nothing; the mask tensor never ships to the device.

Eta values and T' are baked into the program at build time (the program is
rebuilt per kernel() call, so any inputs work).
"""

import math
import os
import sys

import numpy as np

if "/opt/trn_rl_repo" not in sys.path:
    sys.path.insert(0, "/opt/trn_rl_repo")

from contextlib import ExitStack

import concourse.bass as bass
import concourse.tile as tile
from concourse import bacc, mybir
from concourse.bass_utils import run_bass_kernel_spmd

F32 = mybir.dt.float32
F16 = mybir.dt.float16
I32 = mybir.dt.int32
Act = mybir.ActivationFunctionType
Alu = mybir.AluOpType

B, N, T = 8, 512, 512
P = 128                    # SBUF partitions
NCH = N // P               # 4 n-chunks
ZETAS = (1, 2, 4, 16)
NE = 8                     # etas
NZ = 4

# dtype of the contraction inputs (E and G tiles). f16 doubles the DVE
# product throughput; error ~3e-4 of absmax. F32 is the safe mode.
PROD_DT = F16

# Contraction split over the 32 (e,z) pairs. Every pair materializes a
# product tile P = E_e*G_z (producer: DVE f16 tensor_tensor at 2x, or
# GpSimd), then reduces each n-chunk's Tp-column block: either one DVE
# grouped tensor_reduce ([P,4,Tp] -> [P,4]) or 4 ACT Copy-with-accum ops.
#   ACT_PAIRS: how many pairs reduce on ACT (rest reduce on DVE)
#   POOL_PRODS: how many products are produced by GpSimd (rest DVE)
ACT_PAIRS = int(os.environ.get("BEHLER_ACT_PAIRS", "13"))
POOL_PRODS = int(os.environ.get("BEHLER_POOL_PRODS", "13"))

# Engine per square-family op: "act" | "dve" | "gps".
SQ_ENGINES = {
    "fij": "act", "fik": "act",            # fc = sin^2
    "sqij": "gps", "sqik": "gps", "sqjk": "gps",
    "x2": "dve", "x4": "dve", "x8": "dve", "x16": "dve",
}


def _build_nc(etas: np.ndarray, Tp: int) -> bass.Bass:
    W = NCH * Tp
    nc = bacc.Bacc("TRN2", target_bir_lowering=False, debug=False, num_devices=B)

    d_rij = nc.dram_tensor("r_ij", [N, Tp], F32, kind="ExternalInput").ap()
    d_rik = nc.dram_tensor("r_ik", [N, Tp], F32, kind="ExternalInput").ap()
    d_rjk = nc.dram_tensor("r_jk", [N, Tp], F32, kind="ExternalInput").ap()
    d_out = nc.dram_tensor("out", [N, NE * 2 * NZ], F32, kind="ExternalOutput").ap()

    # Register the Sin bias constant (pi/2) like Bass.__init__ does for 0/1.
    half_pi = math.pi / 2
    _const_t = nc.alloc_sbuf_tensor("const-f32-half-pi", [128, 1], F32)
    nc.gpsimd.memset(_const_t.ap(), half_pi)
    nc.const_aps.aps[(F32, half_pi)] = _const_t.ap()
    nc.all_engine_barrier()

    with tile.TileContext(nc) as tc, ExitStack() as ctx:
        pool = ctx.enter_context(tc.tile_pool(name="main", bufs=1))

        # tags are physical slots (reserved per tag for the pool's
        # lifetime); tensors with disjoint lifetimes share a slot.
        def mega(slot, sem_name, dt=F32):
            return pool.tile([P, W], dt, tag=slot, name=sem_name)

        def square(dst, src, eng):
            if eng == "act":
                nc.scalar.activation(dst[:], src[:], Act.Square)
            elif eng == "dve":
                nc.vector.tensor_mul(dst[:], src[:], src[:])
            else:
                nc.gpsimd.tensor_mul(dst[:], src[:], src[:])

        # ---- load inputs: chunk c of DRAM rows -> mega cols [c*Tp,(c+1)*Tp) ----
        rij = mega("s0", "rij")
        rik = mega("s1", "rik")
        rjk = mega("s2", "rjk")
        for tl, dr in ((rij, d_rij), (rik, d_rik), (rjk, d_rjk)):
            for c in range(NCH):
                nc.sync.dma_start(
                    out=tl[:, c * Tp:(c + 1) * Tp], in_=dr[c * P:(c + 1) * P, :]
                )

        # ---- fc via sin^2 (ACT, sin table set first) ----
        fijs = mega("s3", "fijs")
        fiks = mega("s4", "fiks")
        nc.scalar.activation(fijs[:], rij[:], Act.Sin,
                             bias=math.pi / 2, scale=math.pi / 12)
        nc.scalar.activation(fiks[:], rik[:], Act.Sin,
                             bias=math.pi / 2, scale=math.pi / 12)
        fij = mega("s5", "fij")
        fik = mega("s6", "fik")
        square(fij, fijs, SQ_ENGINES["fij"])
        square(fik, fiks, SQ_ENGINES["fik"])

        # ---- squares / u / p / numer2 / xq ----
        sqij = mega("s7", "sqij")
        sqik = mega("s8", "sqik")
        sqjk = mega("s9", "sqjk")
        square(sqij, rij, SQ_ENGINES["sqij"])
        square(sqik, rik, SQ_ENGINES["sqik"])
        square(sqjk, rjk, SQ_ENGINES["sqjk"])

        p = mega("s10", "p")
        nc.vector.tensor_mul(p[:], rij[:], rik[:])       # rij, rik dead
        u = mega("s11", "u")
        nc.vector.tensor_add(u[:], sqij[:], sqik[:])     # sqij, sqik dead
        tsub = mega("s7", "tsub")
        nc.vector.tensor_sub(tsub[:], sqjk[:], u[:])     # sqjk dead

        rp = mega("s8", "rp")
        rscr = mega("s3", "rscr")                        # fijs dead
        nc.vector.reciprocal_approx_accurate(out=rp[:], in_=p[:], scratch=rscr[:])

        numer2 = mega("s0", "numer2")
        nc.vector.scalar_tensor_tensor(
            numer2[:], p[:], 2.0, tsub[:], op0=Alu.mult, op1=Alu.add
        )                                                # p, tsub dead
        xq = mega("s1", "xq")
        nc.vector.scalar_tensor_tensor(
            xq[:], rp[:], 0.25, numer2[:], op0=Alu.mult, op1=Alu.mult
        )                                                # rp, numer2 dead

        R = mega("s2", "R")
        nc.vector.tensor_mul(R[:], fij[:], fik[:])       # fij, fik dead

        # ---- xq powers ----
        x2 = mega("s4", "x2")                            # fiks dead
        x4 = mega("s9", "x4")
        x8 = mega("s10", "x8")                           # p dead
        x16 = mega("s7", "x16")                          # tsub dead
        square(x2, xq, SQ_ENGINES["x2"])
        square(x4, x2, SQ_ENGINES["x4"])
        square(x8, x4, SQ_ENGINES["x8"])
        square(x16, x8, SQ_ENGINES["x16"])

        # ---- G_z = R * xq^z  (gpsimd; f16 out) ----
        powers = {1: xq, 2: x2, 4: x4, 16: x16}
        G = {}
        for z in ZETAS:
            G[z] = mega(f"g{z}", f"g{z}", PROD_DT)
            nc.gpsimd.tensor_mul(G[z][:], R[:], powers[z][:])

        # ---- E_e = exp(-eta_e * u)  (ACT, exp table set; f16 out) ----
        E = []
        for e in range(NE):
            te = mega(f"e{e}", f"e{e}", PROD_DT)
            nc.scalar.activation(te[:], u[:], Act.Exp, scale=-float(etas[e]))
            E.append(te)

        # ---- contraction: S[n, (e*NZ+zi)*NCH + c] = sum_t E_e*G_z ----
        S = pool.tile([P, NE * NZ * NCH], F32, tag="S", name="S")
        scr_a = pool.tile([P, Tp], PROD_DT, tag="scr_a", name="scr_a")

        pairs = [(e, zi) for e in range(NE) for zi in range(NZ)]
        # spread ACT-reduced pairs evenly through program order so the
        # ACT queue drains alongside the DVE one
        n_act = max(0, min(len(pairs), ACT_PAIRS))
        act_set = set()
        if n_act:
            step = len(pairs) / n_act
            act_set = {int(i * step) for i in range(n_act)}
        pool_set = set()
        if POOL_PRODS:
            step = len(pairs) / min(len(pairs), POOL_PRODS)
            pool_set = {int(i * step) for i in range(min(len(pairs), POOL_PRODS))}
        for pi, (e, zi) in enumerate(pairs):
            z = ZETAS[zi]
            prod = pool.tile([P, W], PROD_DT, tag="prod", name=f"prod{pi}",
                             bufs=4)
            if pi in pool_set:
                nc.gpsimd.tensor_mul(prod[:], E[e][:], G[z][:])
            else:
                nc.vector.tensor_mul(prod[:], E[e][:], G[z][:])
            base = (e * NZ + zi) * NCH
            if pi in act_set:
                for c in range(NCH):
                    nc.scalar.activation(
                        scr_a[:], prod[:, c * Tp:(c + 1) * Tp], Act.Copy,
                        accum_out=S[:, base + c:base + c + 1],
                    )
            else:
                nc.vector.tensor_reduce(
                    S[:, base:base + NCH],
                    prod[:].rearrange("p (c t) -> p c t", c=NCH),
                    axis=mybir.AxisListType.X, op=Alu.add,
                )

        # ---- epilogue: out[n, e*8+a], a<4: 2*S ; a>=4: 2^(1+2z)*S ----
        out64 = pool.tile([P, NCH * NE * 2 * NZ], F32, tag="out64", name="out64")
        S_v = S[:].rearrange("p (e z c) -> p e z c", e=NE, z=NZ, c=NCH)
        o_v = out64[:].rearrange("p (c e a) -> p e c a", c=NCH, e=NE, a=2 * NZ)
        for zi, z in enumerate(ZETAS):
            nc.vector.tensor_scalar_mul(o_v[:, :, :, zi], S_v[:, :, zi, :], 2.0)
            nc.vector.tensor_scalar_mul(
                o_v[:, :, :, 4 + zi], S_v[:, :, zi, :], float(2.0 ** (1 + 2 * z))
            )

        for c in range(NCH):
            nc.sync.dma_start(
                out=d_out[c * P:(c + 1) * P, :],
                in_=out64[:, c * (2 * NE * NZ):(c + 1) * (2 * NE * NZ)],
            )

    nc.compile()
    return nc


def _prepare(r_ij, r_ik, r_jk, mask_triples):
    """Compact triples by mask per (b,n); pad to T' with fc-killing r=6."""
    valid = mask_triples != 0
    counts = valid.sum(-1)
    Tp = int(counts.max())
    Tp = max(64, ((Tp + 63) // 64) * 64)
    Tp = min(Tp, T)
    order = np.argsort(~valid, axis=-1, kind="stable")[..., :Tp]
    take = lambda a: np.ascontiguousarray(
        np.take_along_axis(np.asarray(a, dtype=np.float32), order, axis=-1))
    rij, rik, rjk = take(r_ij), take(r_ik), take(r_jk)
    pad = ~np.take_along_axis(valid, order, axis=-1)
    rij[pad] = 6.0
    rik[pad] = 6.0
    rjk[pad] = 6.0
    return rij, rik, rjk, Tp


def kernel(r_ij, r_ik, r_jk, mask_triples, etas):
    mask = np.asarray(mask_triples)
    etas = np.asarray(etas, dtype=np.float32)

    rij, rik, rjk, Tp = _prepare(r_ij, r_ik, r_jk, mask)
    nc = _build_nc(etas, Tp)
    in_maps = [
        {"r_ij": rij[b], "r_ik": rik[b], "r_jk": rjk[b]} for b in range(B)
    ]
    res = run_bass_kernel_spmd(
        nc,
        in_maps,
        core_ids=list(range(B)),
        trace=bool(int(os.environ.get("BEHLER_TRACE", "0"))),
    )
    out = np.stack([res.results[b]["out"] for b in range(B)]).astype(np.float32)
    if getattr(kernel, "_keep_results", False):
        kernel._last_results = res
    return out


# revision 13
# speedup vs baseline: 1.2625x; 1.2617x over previous
"""Behler G3 symmetry-function kernel for Trainium2 (8 NeuronCores).

Math (per batch b, atom n; reduction over triples t):
    fc(r)      = 0.5*(cos(pi*r/6)+1) = sin(pi*r/12 + pi/2)^2        (r < 6 always)
    u          = r_ij^2 + r_ik^2
    1 - cos_t  = (r_jk^2 - (r_ij-r_ik)^2) / (2 r_ij r_ik)
               = numer2 / (2 p),  numer2 = 2p + (r_jk^2 - u), p = r_ij r_ik
    xq         = (1-cos_t)/2 = numer2 * (1/p) * 0.25                 in [0,1]
    R          = fc(r_ij)*fc(r_ik)
    G_z        = R * xq^z                       z in {1,2,4,16}
    E_e        = exp(-eta_e * u)                e in 0..7
    S[n,e,z]   = sum_t E_e * G_z
    out[n, e*8+a] = 2*S[e,a]              for a<4
                  = 2^(1+2*z)*S[e,a-4]    for a>=4   (z = zeta[a-4])
  (reference ang coeffs 2^(1±z) on (1-cos)^z equal these on xq^z.)

Sharding: data-parallel over batch: core b handles batch b. No collectives.

Host-side prep inside kernel(): the t-reduction is permutation-invariant, so
triples are compacted by mask per (b,n) — valid triples first, padded to the
max valid count (T'). Padding entries use r=6.0, where fc(6)=0 exactly, so
they contribute nothing; the mask tensor never ships to the device.

Eta values and T' are baked into the program at build time (the program is
rebuilt per kernel() call, so any inputs work).
"""

import math
import os
import sys

import numpy as np

if "/opt/trn_rl_repo" not in sys.path:
    sys.path.insert(0, "/opt/trn_rl_repo")

from contextlib import ExitStack

import concourse.bass as bass
import concourse.tile as tile
from concourse import bacc, mybir
from concourse.bass_utils import run_bass_kernel_spmd

F32 = mybir.dt.float32
F16 = mybir.dt.float16
I32 = mybir.dt.int32
Act = mybir.ActivationFunctionType
Alu = mybir.AluOpType

B, N, T = 8, 512, 512
P = 128                    # SBUF partitions
NCH = N // P               # 4 n-chunks
ZETAS = (1, 2, 4, 16)
NE = 8                     # etas
NZ = 4

# dtype of the contraction inputs (E and G tiles). f16 doubles the DVE
# product throughput; error ~3e-4 of absmax. F32 is the safe mode.
PROD_DT = F16

# Contraction split over the 32 (e,z) pairs. Every pair materializes a
# product tile P = E_e*G_z (producer: DVE f16 tensor_tensor at 2x, or
# GpSimd), then reduces each n-chunk's Tp-column block: either one DVE
# grouped tensor_reduce ([P,4,Tp] -> [P,4]) or 4 ACT Copy-with-accum ops.
#   ACT_PAIRS: how many pairs reduce on ACT (rest reduce on DVE)
#   POOL_PRODS: how many products are produced by GpSimd (rest DVE)
ACT_PAIRS = int(os.environ.get("BEHLER_ACT_PAIRS", "16"))
POOL_PRODS = int(os.environ.get("BEHLER_POOL_PRODS", "0"))

# Engine per square-family op: "act" | "dve" | "gps".
SQ_ENGINES = {
    "fij": "act", "fik": "act",            # fc = sin^2
    "sqij": "act", "sqik": "act", "sqjk": "act",
    "x2": "dve", "x4": "dve", "x8": "dve", "x16": "dve",
}


def _build_nc(etas: np.ndarray, Tp: int) -> bass.Bass:
    W = NCH * Tp
    nc = bacc.Bacc("TRN2", target_bir_lowering=False, debug=False, num_devices=B)

    d_rij = nc.dram_tensor("r_ij", [N, Tp], F32, kind="ExternalInput").ap()
    d_rik = nc.dram_tensor("r_ik", [N, Tp], F32, kind="ExternalInput").ap()
    d_rjk = nc.dram_tensor("r_jk", [N, Tp], F32, kind="ExternalInput").ap()
    d_out = nc.dram_tensor("out", [N, NE * 2 * NZ], F32, kind="ExternalOutput").ap()

    with tile.TileContext(nc) as tc, ExitStack() as ctx:
        pool = ctx.enter_context(tc.tile_pool(name="main", bufs=1))

        # tags are physical slots (reserved per tag for the pool's
        # lifetime); tensors with disjoint lifetimes share a slot.
        def mega(slot, sem_name, dt=F32):
            return pool.tile([P, W], dt, tag=slot, name=sem_name)

        def square(dst, src, eng):
            if eng == "act":
                nc.scalar.activation(dst[:], src[:], Act.Square)
            elif eng == "dve":
                nc.vector.tensor_mul(dst[:], src[:], src[:])
            else:
                nc.gpsimd.tensor_mul(dst[:], src[:], src[:])

        # ---- load inputs: chunk c of DRAM rows -> mega cols [c*Tp,(c+1)*Tp) ----
        rij = mega("s0", "rij")
        rik = mega("s1", "rik")
        rjk = mega("s2", "rjk")
        for tl, dr in ((rij, d_rij), (rik, d_rik), (rjk, d_rjk)):
            for c in range(NCH):
                nc.sync.dma_start(
                    out=tl[:, c * Tp:(c + 1) * Tp], in_=dr[c * P:(c + 1) * P, :]
                )

        # ---- fc = 1 - sin^2(pi*r/12)  (= cos^2(pi*r/12), no bias const) ----
        fijs = mega("s3", "fijs")
        fiks = mega("s4", "fiks")
        nc.scalar.activation(fijs[:], rij[:], Act.Sin, scale=math.pi / 12)
        nc.scalar.activation(fiks[:], rik[:], Act.Sin, scale=math.pi / 12)
        sijq = mega("s5", "sijq")
        sikq = mega("s6", "sikq")
        square(sijq, fijs, SQ_ENGINES["fij"])
        square(sikq, fiks, SQ_ENGINES["fik"])
        fij = mega("s3", "fij")       # fijs dead
        fik = mega("s4", "fik")       # fiks dead
        nc.vector.tensor_scalar(fij[:], sijq[:], -1.0, 1.0,
                                op0=Alu.mult, op1=Alu.add)
        nc.vector.tensor_scalar(fik[:], sikq[:], -1.0, 1.0,
                                op0=Alu.mult, op1=Alu.add)

        # ---- squares / u / p / numer2 / xq ----
        sqij = mega("s7", "sqij")
        sqik = mega("s8", "sqik")
        sqjk = mega("s9", "sqjk")
        square(sqij, rij, SQ_ENGINES["sqij"])
        square(sqik, rik, SQ_ENGINES["sqik"])
        square(sqjk, rjk, SQ_ENGINES["sqjk"])

        p = mega("s10", "p")
        nc.vector.tensor_mul(p[:], rij[:], rik[:])       # rij, rik dead
        u = mega("s11", "u")
        nc.vector.tensor_add(u[:], sqij[:], sqik[:])     # sqij, sqik dead
        tsub = mega("s7", "tsub")
        nc.vector.tensor_sub(tsub[:], sqjk[:], u[:])     # sqjk dead

        rp = mega("s8", "rp")
        rscr = mega("s5", "rscr")                        # sijq dead
        nc.vector.reciprocal_approx_accurate(out=rp[:], in_=p[:], scratch=rscr[:])

        numer2 = mega("s0", "numer2")
        nc.vector.scalar_tensor_tensor(
            numer2[:], p[:], 2.0, tsub[:], op0=Alu.mult, op1=Alu.add
        )                                                # p, tsub dead
        xq = mega("s1", "xq")
        nc.vector.scalar_tensor_tensor(
            xq[:], rp[:], 0.25, numer2[:], op0=Alu.mult, op1=Alu.mult
        )                                                # rp, numer2 dead

        R = mega("s2", "R")
        nc.vector.tensor_mul(R[:], fij[:], fik[:])       # fij, fik dead

        # ---- xq powers ----
        x2 = mega("s6", "x2")                            # sikq dead
        x4 = mega("s9", "x4")
        x8 = mega("s10", "x8")                           # p dead
        x16 = mega("s7", "x16")                          # tsub dead
        square(x2, xq, SQ_ENGINES["x2"])
        square(x4, x2, SQ_ENGINES["x4"])
        square(x8, x4, SQ_ENGINES["x8"])
        square(x16, x8, SQ_ENGINES["x16"])

        # ---- G_z = R * xq^z  (gpsimd; f16 out) ----
        powers = {1: xq, 2: x2, 4: x4, 16: x16}
        G = {}
        for z in ZETAS:
            G[z] = mega(f"g{z}", f"g{z}", PROD_DT)
            nc.vector.tensor_mul(G[z][:], R[:], powers[z][:])

        # ---- E_e = exp(-eta_e * u)  (ACT, exp table set; f16 out) ----
        E = []
        for e in range(NE):
            te = mega(f"e{e}", f"e{e}", PROD_DT)
            nc.scalar.activation(te[:], u[:], Act.Exp, scale=-float(etas[e]))
            E.append(te)

        # ---- contraction: S[n, (e*NZ+zi)*NCH + c] = sum_t E_e*G_z ----
        S = pool.tile([P, NE * NZ * NCH], F32, tag="S", name="S")
        scr_a = pool.tile([P, Tp], PROD_DT, tag="scr_a", name="scr_a")

        pairs = [(e, zi) for e in range(NE) for zi in range(NZ)]
        # spread ACT-reduced pairs evenly through program order so the
        # ACT queue drains alongside the DVE one
        n_act = max(0, min(len(pairs), ACT_PAIRS))
        act_set = set()
        if n_act:
            step = len(pairs) / n_act
            act_set = {int(i * step) for i in range(n_act)}
        pool_set = set()
        if POOL_PRODS:
            step = len(pairs) / min(len(pairs), POOL_PRODS)
            pool_set = {int(i * step) for i in range(min(len(pairs), POOL_PRODS))}
        for pi, (e, zi) in enumerate(pairs):
            z = ZETAS[zi]
            prod = pool.tile([P, W], PROD_DT, tag="prod", name=f"prod{pi}",
                             bufs=4)
            if pi in pool_set:
                nc.gpsimd.tensor_mul(prod[:], E[e][:], G[z][:])
            else:
                nc.vector.tensor_mul(prod[:], E[e][:], G[z][:])
            base = (e * NZ + zi) * NCH
            if pi in act_set:
                for c in range(NCH):
                    nc.scalar.activation(
                        scr_a[:], prod[:, c * Tp:(c + 1) * Tp], Act.Copy,
                        accum_out=S[:, base + c:base + c + 1],
                    )
            else:
                nc.vector.tensor_reduce(
                    S[:, base:base + NCH],
                    prod[:].rearrange("p (c t) -> p c t", c=NCH),
                    axis=mybir.AxisListType.X, op=Alu.add,
                )

        # ---- epilogue: out[n, e*8+a], a<4: 2*S ; a>=4: 2^(1+2z)*S ----
        out64 = pool.tile([P, NCH * NE * 2 * NZ], F32, tag="out64", name="out64")
        S_v = S[:].rearrange("p (e z c) -> p e z c", e=NE, z=NZ, c=NCH)
        o_v = out64[:].rearrange("p (c e a) -> p e c a", c=NCH, e=NE, a=2 * NZ)
        for zi, z in enumerate(ZETAS):
            nc.vector.tensor_scalar_mul(o_v[:, :, :, zi], S_v[:, :, zi, :], 2.0)
            nc.vector.tensor_scalar_mul(
                o_v[:, :, :, 4 + zi], S_v[:, :, zi, :], float(2.0 ** (1 + 2 * z))
            )

        for c in range(NCH):
            nc.sync.dma_start(
                out=d_out[c * P:(c + 1) * P, :],
                in_=out64[:, c * (2 * NE * NZ):(c + 1) * (2 * NE * NZ)],
            )

    nc.compile()
    return nc


def _prepare(r_ij, r_ik, r_jk, mask_triples):
    """Compact triples by mask per (b,n); pad to T' with fc-killing r=6."""
    valid = mask_triples != 0
    counts = valid.sum(-1)
    Tp = int(counts.max())
    Tp = max(64, ((Tp + 63) // 64) * 64)
    Tp = min(Tp, T)
    order = np.argsort(~valid, axis=-1, kind="stable")[..., :Tp]
    take = lambda a: np.ascontiguousarray(
        np.take_along_axis(np.asarray(a, dtype=np.float32), order, axis=-1))
    rij, rik, rjk = take(r_ij), take(r_ik), take(r_jk)
    pad = ~np.take_along_axis(valid, order, axis=-1)
    rij[pad] = 6.0
    rik[pad] = 6.0
    rjk[pad] = 6.0
    return rij, rik, rjk, Tp


def kernel(r_ij, r_ik, r_jk, mask_triples, etas):
    mask = np.asarray(mask_triples)
    etas = np.asarray(etas, dtype=np.float32)

    rij, rik, rjk, Tp = _prepare(r_ij, r_ik, r_jk, mask)
    nc = _build_nc(etas, Tp)
    in_maps = [
        {"r_ij": rij[b], "r_ik": rik[b], "r_jk": rjk[b]} for b in range(B)
    ]
    res = run_bass_kernel_spmd(
        nc,
        in_maps,
        core_ids=list(range(B)),
        trace=bool(int(os.environ.get("BEHLER_TRACE", "0"))),
    )
    out = np.stack([res.results[b]["out"] for b in range(B)]).astype(np.float32)
    if getattr(kernel, "_keep_results", False):
        kernel._last_results = res
    return out
